# revision 2
# baseline (speedup 1.0000x reference)
"""BN1d-with-filtered-moments Bass kernel for 8 trn2 NeuronCores.

Computes, over the full (128, 524288) f32 input x:
  mean/var (ddof=1) -> mask = |(x-mean)/sqrt(var+eps)| < 4 (strict)
  masked mean/var (ddof=1 over selected) -> EMA step (alpha=0.9 from 0/1)
  out = gamma * (x - run_mean) / sqrt(run_var + eps) + beta

Sharding: data-parallel over the batch axis (16 rows per core). Each core
computes per-shard partial sums; two tiny AllReduces combine them; the
affine transform is fully local.

Per-core pipeline (shard viewed as [128, 65536] f32):
  pass 1: stream x: DVE ts(mult 1, accum add) -> sum(x); ACT Square(accum)
          -> sum(x^2). ones-matmul folds partitions; AllReduce #1; thresholds
          lo/hi = mean -/+ 4*sqrt(var+eps) (ACT sqrt + one Newton step).
  pass 2: stream x: DVE clip c = min(max(x,lo),hi); DVE ts(mult 1,accum) ->
          sum(c); DVE is_le/is_ge (accum) -> exact outlier counts; ACT
          Square(c, accum) -> sum(c^2). AllReduce #2; masked moments via
            sum_masked(x)   = sum(c) - lo*n_lo - hi*n_hi
            sum_masked(x^2) = sum(c^2) - lo^2*n_lo - hi^2*n_hi
            cnt             = n - n_lo - n_hi
  pass 3: stream x: DVE ts (x*a)+b -> out, with a = gamma/sqrt(run_var+eps),
          b = beta - run_mean*a.
"""

import numpy as np

import concourse.bass as bass
import concourse.bacc as bacc
import concourse.mybir as mybir
import concourse.tile as tile
from concourse.bass_utils import run_bass_kernel_spmd

F32 = mybir.dt.float32
ALU = mybir.AluOpType
ACTF = mybir.ActivationFunctionType

N_CORES = 8
P = 128

# Full problem geometry (hardcoded; the grading harness provides no spec files)
FULL_ROWS = 128
FULL_COLS = 524288
CORE_ROWS = FULL_ROWS // N_CORES          # 16 rows per core
CORE_ELEMS = CORE_ROWS * FULL_COLS        # 8388608
F_FULL = CORE_ELEMS // P                  # 65536 per partition
CF_FULL = 2048                            # chunk free-dim (1 MiB DMA tiles)

THRES = 4.0
ALPHA = 0.9
EPS = 1e-10


def build_bass(f_per_part: int, cf: int, n_cores: int = N_CORES,
               xt_bufs: int = 8):
    """Build the SPMD Bass program for a per-core shard of [P, f_per_part]."""
    assert f_per_part % cf == 0
    nch = f_per_part // cf
    n_total = float(n_cores * P * f_per_part)

    nc = bacc.Bacc(
        "TRN2",
        target_bir_lowering=False,
        debug=False,
        num_devices=n_cores,
    )

    x = nc.dram_tensor("x", [P, f_per_part], F32, kind="ExternalInput")
    gamma = nc.dram_tensor("gamma", [1, 1], F32, kind="ExternalInput")
    beta = nc.dram_tensor("beta", [1, 1], F32, kind="ExternalInput")
    out = nc.dram_tensor("out", [P, f_per_part], F32, kind="ExternalOutput")

    groups = [list(range(n_cores))]

    with tile.TileContext(nc) as tc:
        with (
            tc.tile_pool(name="xs", bufs=xt_bufs) as xpool,
            tc.tile_pool(name="cs", bufs=3) as cpool,
            tc.tile_pool(name="scrv", bufs=3) as svpool,
            tc.tile_pool(name="scra", bufs=2) as sapool,
            tc.tile_pool(name="os", bufs=3) as opool,
            tc.tile_pool(name="accs", bufs=1) as apool,
            tc.tile_pool(name="small", bufs=1) as smpool,
            tc.tile_pool(name="psum", bufs=1, space="PSUM") as pspool,
            tc.tile_pool(name="dram", bufs=1, space="DRAM") as drpool,
        ):
            # ---- constants / small tiles -------------------------------
            ones = smpool.tile([P, 1], F32, tag="ones", name="ones")
            nc.vector.memset(ones[:], 1.0)

            gsb = smpool.tile([1, 1], F32, tag="gsb", name="gsb")
            bsb = smpool.tile([1, 1], F32, tag="bsb", name="bsb")
            nc.sync.dma_start(out=gsb[:], in_=gamma[:])
            nc.sync.dma_start(out=bsb[:], in_=beta[:])
            gamma_b = smpool.tile([P, 1], F32, tag="gamma_b", name="gamma_b")
            beta_b = smpool.tile([P, 1], F32, tag="beta_b", name="beta_b")
            nc.gpsimd.partition_broadcast(gamma_b[:], gsb[:])
            nc.gpsimd.partition_broadcast(beta_b[:], bsb[:])

            # per-chunk accumulator buffers
            acc_sx = apool.tile([P, nch], F32, tag="acc_sx", name="acc_sx")
            acc_sxx = apool.tile([P, nch], F32, tag="acc_sxx", name="acc_sxx")
            acc_sc = apool.tile([P, nch], F32, tag="acc_sc", name="acc_sc")
            acc_scc = apool.tile([P, nch], F32, tag="acc_scc", name="acc_scc")
            acc_nlo = apool.tile([P, nch], F32, tag="acc_nlo", name="acc_nlo")
            acc_nhi = apool.tile([P, nch], F32, tag="acc_nhi", name="acc_nhi")

            # ================= pass 1: sum(x), sum(x^2) =================
            for k in range(nch):
                xt = xpool.tile([P, cf], F32, tag="xt", name="xt")
                nc.sync.dma_start(out=xt[:], in_=x[:, k * cf:(k + 1) * cf])
                sv = svpool.tile([P, cf], F32, tag="scrv", name="scrv")
                nc.vector.tensor_scalar(
                    out=sv[:], in0=xt[:], scalar1=1.0, scalar2=None,
                    op0=ALU.mult, op1=ALU.add,
                    accum_out=acc_sx[:, k:k + 1],
                )
                sa = sapool.tile([P, cf], F32, tag="scra", name="scra")
                nc.scalar.activation(
                    out=sa[:], in_=xt[:], func=ACTF.Square,
                    accum_out=acc_sxx[:, k:k + 1],
                )

            # ---- fold partials, AllReduce #1 ---------------------------
            vals1 = smpool.tile([P, 2], F32, tag="vals1", name="vals1")
            nc.vector.reduce_sum(out=vals1[:, 0:1], in_=acc_sx[:],
                                 axis=mybir.AxisListType.X)
            nc.vector.reduce_sum(out=vals1[:, 1:2], in_=acc_sxx[:],
                                 axis=mybir.AxisListType.X)
            ps1 = pspool.tile([1, 2], F32, tag="ps1", name="ps1")
            nc.tensor.matmul(out=ps1[:], lhsT=ones[:], rhs=vals1[:],
                             start=True, stop=True)
            loc1 = smpool.tile([1, 8], F32, tag="loc1", name="loc1")
            nc.vector.memset(loc1[:], 0.0)
            nc.vector.tensor_copy(out=loc1[:, 0:2], in_=ps1[:])

            ar1_in = drpool.tile([1, 8], F32, tag="ar1_in", name="ar1_in")
            ar1_out = drpool.tile([1, 8], F32, tag="ar1_out", name="ar1_out")
            nc.sync.dma_start(out=ar1_in[:], in_=loc1[:])
            nc.gpsimd.collective_compute(
                "AllReduce", ALU.add, replica_groups=groups,
                ins=[ar1_in.opt()], outs=[ar1_out.opt()],
            )
            g1 = smpool.tile([1, 8], F32, tag="g1", name="g1")
            nc.sync.dma_start(out=g1[:], in_=ar1_out[:])
            gb1 = smpool.tile([P, 8], F32, tag="gb1", name="gb1")
            nc.gpsimd.partition_broadcast(gb1[:], g1[:])

            # ---- thresholds lo/hi (all [P,1], replicated rows) ---------
            def s_tile(tag):
                return smpool.tile([P, 1], F32, tag=tag, name=tag)

            s1g = gb1[:, 0:1]
            s2g = gb1[:, 1:2]
            mean = s_tile("mean")
            nc.vector.tensor_scalar(out=mean[:], in0=s1g, scalar1=1.0 / n_total,
                                    scalar2=None, op0=ALU.mult)
            t1 = s_tile("t1")
            nc.vector.tensor_tensor(out=t1[:], in0=s1g, in1=mean[:], op=ALU.mult)
            t2 = s_tile("t2")
            nc.vector.tensor_tensor(out=t2[:], in0=s2g, in1=t1[:], op=ALU.subtract)
            sig2 = s_tile("sig2")
            nc.vector.tensor_scalar(out=sig2[:], in0=t2[:],
                                    scalar1=1.0 / (n_total - 1.0), scalar2=EPS,
                                    op0=ALU.mult, op1=ALU.add)
            # sd = sqrt(sig2), Newton-refined: sd = 0.5*(sd0 + sig2/sd0)
            sd0 = s_tile("sd0")
            nc.scalar.sqrt(sd0[:], sig2[:])
            r0 = s_tile("r0")
            nc.vector.reciprocal(r0[:], sd0[:])
            t3 = s_tile("t3")
            nc.vector.tensor_tensor(out=t3[:], in0=sig2[:], in1=r0[:], op=ALU.mult)
            t4 = s_tile("t4")
            nc.vector.tensor_tensor(out=t4[:], in0=sd0[:], in1=t3[:], op=ALU.add)
            s4 = s_tile("s4")
            nc.vector.tensor_scalar(out=s4[:], in0=t4[:], scalar1=0.5 * THRES,
                                    scalar2=None, op0=ALU.mult)
            lo = s_tile("lo")
            nc.vector.tensor_tensor(out=lo[:], in0=mean[:], in1=s4[:],
                                    op=ALU.subtract)
            hi = s_tile("hi")
            nc.vector.tensor_tensor(out=hi[:], in0=mean[:], in1=s4[:], op=ALU.add)

            # ===== pass 2: sum(c), sum(c^2), n_lo, n_hi =================
            for k in range(nch):
                xt = xpool.tile([P, cf], F32, tag="xt", name="xt")
                nc.sync.dma_start(out=xt[:], in_=x[:, k * cf:(k + 1) * cf])
                ct = cpool.tile([P, cf], F32, tag="ct", name="ct")
                nc.vector.tensor_scalar(
                    out=ct[:], in0=xt[:], scalar1=lo[:, 0:1], scalar2=hi[:, 0:1],
                    op0=ALU.max, op1=ALU.min,
                )
                sv = svpool.tile([P, cf], F32, tag="scrv", name="scrv")
                nc.vector.tensor_scalar(
                    out=sv[:], in0=ct[:], scalar1=1.0, scalar2=None,
                    op0=ALU.mult, op1=ALU.add,
                    accum_out=acc_sc[:, k:k + 1],
                )
                sv2 = svpool.tile([P, cf], F32, tag="scrv", name="scrv")
                nc.vector.tensor_scalar(
                    out=sv2[:], in0=xt[:], scalar1=lo[:, 0:1], scalar2=None,
                    op0=ALU.is_le, op1=ALU.add,
                    accum_out=acc_nlo[:, k:k + 1],
                )
                sv3 = svpool.tile([P, cf], F32, tag="scrv", name="scrv")
                nc.vector.tensor_scalar(
                    out=sv3[:], in0=xt[:], scalar1=hi[:, 0:1], scalar2=None,
                    op0=ALU.is_ge, op1=ALU.add,
                    accum_out=acc_nhi[:, k:k + 1],
                )
                sa = sapool.tile([P, cf], F32, tag="scra", name="scra")
                nc.scalar.activation(
                    out=sa[:], in_=ct[:], func=ACTF.Square,
                    accum_out=acc_scc[:, k:k + 1],
                )

            # ---- fold partials, AllReduce #2 ---------------------------
            vals2 = smpool.tile([P, 4], F32, tag="vals2", name="vals2")
            nc.vector.reduce_sum(out=vals2[:, 0:1], in_=acc_sc[:],
                                 axis=mybir.AxisListType.X)
            nc.vector.reduce_sum(out=vals2[:, 1:2], in_=acc_scc[:],
                                 axis=mybir.AxisListType.X)
            nc.vector.reduce_sum(out=vals2[:, 2:3], in_=acc_nlo[:],
                                 axis=mybir.AxisListType.X)
            nc.vector.reduce_sum(out=vals2[:, 3:4], in_=acc_nhi[:],
                                 axis=mybir.AxisListType.X)
            ps2 = pspool.tile([1, 4], F32, tag="ps2", name="ps2")
            nc.tensor.matmul(out=ps2[:], lhsT=ones[:], rhs=vals2[:],
                             start=True, stop=True)
            loc2 = smpool.tile([1, 8], F32, tag="loc2", name="loc2")
            nc.vector.memset(loc2[:], 0.0)
            nc.vector.tensor_copy(out=loc2[:, 0:4], in_=ps2[:])

            ar2_in = drpool.tile([1, 8], F32, tag="ar2_in", name="ar2_in")
            ar2_out = drpool.tile([1, 8], F32, tag="ar2_out", name="ar2_out")
            nc.sync.dma_start(out=ar2_in[:], in_=loc2[:])
            nc.gpsimd.collective_compute(
                "AllReduce", ALU.add, replica_groups=groups,
                ins=[ar2_in.opt()], outs=[ar2_out.opt()],
            )
            g2 = smpool.tile([1, 8], F32, tag="g2", name="g2")
            nc.sync.dma_start(out=g2[:], in_=ar2_out[:])
            gb2 = smpool.tile([P, 8], F32, tag="gb2", name="gb2")
            nc.gpsimd.partition_broadcast(gb2[:], g2[:])

            # ---- masked moments -> EMA -> affine coefficients ----------
            sc_g = gb2[:, 0:1]
            scc_g = gb2[:, 1:2]
            nlo_g = gb2[:, 2:3]
            nhi_g = gb2[:, 3:4]

            u = s_tile("u")
            nc.vector.tensor_tensor(out=u[:], in0=nlo_g, in1=nhi_g, op=ALU.add)
            cnt = s_tile("cnt")
            nc.vector.tensor_scalar(out=cnt[:], in0=u[:], scalar1=n_total,
                                    scalar2=-1.0, op0=ALU.subtract, op1=ALU.mult)
            w1 = s_tile("w1")
            nc.vector.tensor_tensor(out=w1[:], in0=lo[:], in1=nlo_g, op=ALU.mult)
            w2 = s_tile("w2")
            nc.vector.tensor_tensor(out=w2[:], in0=hi[:], in1=nhi_g, op=ALU.mult)
            w3 = s_tile("w3")
            nc.vector.tensor_tensor(out=w3[:], in0=w1[:], in1=w2[:], op=ALU.add)
            s1m = s_tile("s1m")
            nc.vector.tensor_tensor(out=s1m[:], in0=sc_g, in1=w3[:],
                                    op=ALU.subtract)
            lo2 = s_tile("lo2")
            nc.vector.tensor_tensor(out=lo2[:], in0=lo[:], in1=lo[:], op=ALU.mult)
            hi2 = s_tile("hi2")
            nc.vector.tensor_tensor(out=hi2[:], in0=hi[:], in1=hi[:], op=ALU.mult)
            v1 = s_tile("v1")
            nc.vector.tensor_tensor(out=v1[:], in0=lo2[:], in1=nlo_g, op=ALU.mult)
            v2 = s_tile("v2")
            nc.vector.tensor_tensor(out=v2[:], in0=hi2[:], in1=nhi_g, op=ALU.mult)
            v3 = s_tile("v3")
            nc.vector.tensor_tensor(out=v3[:], in0=v1[:], in1=v2[:], op=ALU.add)
            s2m = s_tile("s2m")
            nc.vector.tensor_tensor(out=s2m[:], in0=scc_g, in1=v3[:],
                                    op=ALU.subtract)

            rc = s_tile("rc")
            nc.vector.reciprocal(rc[:], cnt[:])
            pmean = s_tile("pmean")
            nc.vector.tensor_tensor(out=pmean[:], in0=s1m[:], in1=rc[:],
                                    op=ALU.mult)
            pt = s_tile("pt")
            nc.vector.tensor_tensor(out=pt[:], in0=pmean[:], in1=s1m[:],
                                    op=ALU.mult)
            pt2 = s_tile("pt2")
            nc.vector.tensor_tensor(out=pt2[:], in0=s2m[:], in1=pt[:],
                                    op=ALU.subtract)
            cm1 = s_tile("cm1")
            nc.vector.tensor_scalar(out=cm1[:], in0=cnt[:], scalar1=-1.0,
                                    scalar2=None, op0=ALU.add)
            rc1 = s_tile("rc1")
            nc.vector.reciprocal(rc1[:], cm1[:])
            pvar = s_tile("pvar")
            nc.vector.tensor_tensor(out=pvar[:], in0=pt2[:], in1=rc1[:],
                                    op=ALU.mult)

            runm = s_tile("runm")
            nc.vector.tensor_scalar(out=runm[:], in0=pmean[:],
                                    scalar1=1.0 - ALPHA, scalar2=None,
                                    op0=ALU.mult)
            runv = s_tile("runv")
            nc.vector.tensor_scalar(out=runv[:], in0=pvar[:],
                                    scalar1=1.0 - ALPHA, scalar2=ALPHA,
                                    op0=ALU.mult, op1=ALU.add)
            q = s_tile("q")
            nc.vector.tensor_scalar(out=q[:], in0=runv[:], scalar1=EPS,
                                    scalar2=None, op0=ALU.add)
            # rstd = 1/sqrt(q) = refined_sqrt(q) * (1/q)
            qs0 = s_tile("qs0")
            nc.scalar.sqrt(qs0[:], q[:])
            qr0 = s_tile("qr0")
            nc.vector.reciprocal(qr0[:], qs0[:])
            qt = s_tile("qt")
            nc.vector.tensor_tensor(out=qt[:], in0=q[:], in1=qr0[:], op=ALU.mult)
            qt2 = s_tile("qt2")
            nc.vector.tensor_tensor(out=qt2[:], in0=qs0[:], in1=qt[:], op=ALU.add)
            sdr = s_tile("sdr")
            nc.vector.tensor_scalar(out=sdr[:], in0=qt2[:], scalar1=0.5,
                                    scalar2=None, op0=ALU.mult)
            rq = s_tile("rq")
            nc.vector.reciprocal(rq[:], q[:])
            rstd = s_tile("rstd")
            nc.vector.tensor_tensor(out=rstd[:], in0=sdr[:], in1=rq[:],
                                    op=ALU.mult)
            a_co = s_tile("a_co")
            nc.vector.tensor_tensor(out=a_co[:], in0=rstd[:], in1=gamma_b[:],
                                    op=ALU.mult)
            rma = s_tile("rma")
            nc.vector.tensor_tensor(out=rma[:], in0=runm[:], in1=a_co[:],
                                    op=ALU.mult)
            b_co = s_tile("b_co")
            nc.vector.tensor_tensor(out=b_co[:], in0=beta_b[:], in1=rma[:],
                                    op=ALU.subtract)

            # ================= pass 3: out = a*x + b ====================
            for k in range(nch):
                xt = xpool.tile([P, cf], F32, tag="xt", name="xt")
                nc.sync.dma_start(out=xt[:], in_=x[:, k * cf:(k + 1) * cf])
                ot = opool.tile([P, cf], F32, tag="ot", name="ot")
                nc.vector.tensor_scalar(
                    out=ot[:], in0=xt[:], scalar1=a_co[:, 0:1],
                    scalar2=b_co[:, 0:1], op0=ALU.mult, op1=ALU.add,
                )
                nc.sync.dma_start(out=out[:, k * cf:(k + 1) * cf], in_=ot[:])

    nc.compile()
    return nc


_BUILT = {}


def _get_built(f_per_part, cf, n_cores=N_CORES):
    key = (f_per_part, cf, n_cores)
    if key not in _BUILT:
        _BUILT[key] = build_bass(f_per_part, cf, n_cores)
    return _BUILT[key]


def run(xorig: np.ndarray, gamma: np.ndarray, beta: np.ndarray,
        f_per_part: int = F_FULL, cf: int = CF_FULL, **spmd_kwargs):
    """Shard, run on 8 cores, gather. Returns (output, BassKernelResults)."""
    xorig = np.ascontiguousarray(np.asarray(xorig, dtype=np.float32))
    rows, cols = xorig.shape
    assert rows % N_CORES == 0
    g = np.asarray(gamma, dtype=np.float32).reshape(1, 1)
    b = np.asarray(beta, dtype=np.float32).reshape(1, 1)

    nc = _get_built(f_per_part, cf)

    shard_rows = rows // N_CORES
    in_maps = []
    for i in range(N_CORES):
        shard = xorig[i * shard_rows:(i + 1) * shard_rows].reshape(P, f_per_part)
        in_maps.append({"x": shard, "gamma": g, "beta": b})

    res = run_bass_kernel_spmd(nc, in_maps, core_ids=list(range(N_CORES)),
                               **spmd_kwargs)
    outs = [res.results[i]["out"].reshape(shard_rows, cols)
            for i in range(N_CORES)]
    return np.concatenate(outs, axis=0), res


def kernel(xorig, gamma, beta):
    out, _ = run(np.asarray(xorig), np.asarray(gamma), np.asarray(beta))
    return out


# revision 3
# speedup vs baseline: 1.2770x; 1.2770x over previous
"""BN1d-with-filtered-moments Bass kernel for 8 trn2 NeuronCores.

Computes, over the full (128, 524288) f32 input x:
  mean/var (ddof=1) -> mask = |(x-mean)/sqrt(var+eps)| < 4 (strict)
  masked mean/var (ddof=1 over selected) -> EMA step (alpha=0.9 from 0/1)
  out = gamma * (x - run_mean) / sqrt(run_var + eps) + beta

Sharding: data-parallel over the batch axis (16 rows per core). Each core
computes per-shard partial sums; two tiny AllReduces combine them; the
affine transform is fully local.

Per-core pipeline (shard viewed as [128, 65536] f32). All wide reductions
run on the TensorEngine as ones-vector matmuls accumulating into PSUM
(bf16 moving operand, fp32 accumulate); DVE does clip/compares/one cast;
ACT does squares and the final affine. This keeps every engine under the
HBM roofline (~3 reads + 1 write of the shard).

  pass 1: DVE cast x->bf16; ACT Square(x)->bf16. PE: sum(x), sum(x^2).
          AllReduce #1 -> thresholds lo/hi = mean -/+ 4*sqrt(var+eps).
  pass 2: DVE clip c=min(max(x,lo),hi)->bf16, is_le/is_ge indicator tiles
          (bf16); ACT Square(c)->bf16. PE: sum(c), sum(c^2), n_lo, n_hi.
          AllReduce #2 -> masked moments:
            sum_m(x)   = sum(c) - lo*n_lo - hi*n_hi
            sum_m(x^2) = sum(c^2) - lo^2*n_lo - hi^2*n_hi
            cnt        = n - n_lo - n_hi
          -> pmean/pvar -> run stats -> a = gamma/sqrt(run_var+eps),
          b = beta - run_mean*a.
  pass 3: ACT Identity(x*a + b) -> out.
"""

import numpy as np

import concourse.bass as bass
import concourse.bacc as bacc
import concourse.mybir as mybir
import concourse.tile as tile
from concourse.bass_utils import run_bass_kernel_spmd

F32 = mybir.dt.float32
BF16 = mybir.dt.bfloat16
ALU = mybir.AluOpType
ACTF = mybir.ActivationFunctionType

N_CORES = 8
P = 128
MM = 512            # psum bank columns per matmul

# Full problem geometry (hardcoded; the grading harness provides no spec files)
FULL_ROWS = 128
FULL_COLS = 524288
CORE_ROWS = FULL_ROWS // N_CORES          # 16 rows per core
CORE_ELEMS = CORE_ROWS * FULL_COLS        # 8388608
F_FULL = CORE_ELEMS // P                  # 65536 per partition
CF_FULL = 2048                            # chunk free-dim (1 MiB DMA tiles)

THRES = 4.0
ALPHA = 0.9
EPS = 1e-10


def build_bass(f_per_part: int, cf: int, n_cores: int = N_CORES,
               xt_bufs: int = 10):
    """Build the SPMD Bass program for a per-core shard of [P, f_per_part]."""
    assert f_per_part % cf == 0 and cf % MM == 0
    nch = f_per_part // cf
    sub = cf // MM
    n_total = float(n_cores * P * f_per_part)

    nc = bacc.Bacc(
        "TRN2",
        target_bir_lowering=False,
        debug=False,
        num_devices=n_cores,
    )

    x = nc.dram_tensor("x", [P, f_per_part], F32, kind="ExternalInput")
    gamma = nc.dram_tensor("gamma", [1, 1], F32, kind="ExternalInput")
    beta = nc.dram_tensor("beta", [1, 1], F32, kind="ExternalInput")
    out = nc.dram_tensor("out", [P, f_per_part], F32, kind="ExternalOutput")

    groups = [list(range(n_cores))]

    with tile.TileContext(nc) as tc:
        with (
            tc.tile_pool(name="xs", bufs=xt_bufs) as xpool,
            tc.tile_pool(name="bs", bufs=3) as bpool,      # bf16 mm feeds
            tc.tile_pool(name="sq", bufs=3) as sqpool,     # ACT square outs
            tc.tile_pool(name="os", bufs=3) as opool,
            tc.tile_pool(name="small", bufs=1) as smpool,
            tc.tile_pool(name="psum", bufs=1, space="PSUM") as pspool,
            tc.tile_pool(name="dram", bufs=1, space="DRAM") as drpool,
        ):
            # ---- constants / small tiles -------------------------------
            ones_b = smpool.tile([P, 1], BF16, tag="ones_b", name="ones_b")
            nc.vector.memset(ones_b[:], 1.0)

            gsb = smpool.tile([1, 1], F32, tag="gsb", name="gsb")
            bsb = smpool.tile([1, 1], F32, tag="bsb", name="bsb")
            nc.sync.dma_start(out=gsb[:], in_=gamma[:])
            nc.sync.dma_start(out=bsb[:], in_=beta[:])
            gamma_b = smpool.tile([P, 1], F32, tag="gamma_b", name="gamma_b")
            beta_b = smpool.tile([P, 1], F32, tag="beta_b", name="beta_b")
            nc.gpsimd.partition_broadcast(gamma_b[:], gsb[:])
            nc.gpsimd.partition_broadcast(beta_b[:], bsb[:])

            ps_sx = pspool.tile([1, MM], F32, tag="ps_sx", name="ps_sx")
            ps_sxx = pspool.tile([1, MM], F32, tag="ps_sxx", name="ps_sxx")
            ps_sc = pspool.tile([1, MM], F32, tag="ps_sc", name="ps_sc")
            ps_scc = pspool.tile([1, MM], F32, tag="ps_scc", name="ps_scc")
            ps_nlo = pspool.tile([1, MM], F32, tag="ps_nlo", name="ps_nlo")
            ps_nhi = pspool.tile([1, MM], F32, tag="ps_nhi", name="ps_nhi")

            def mm_accum(ps, src, k):
                for j in range(sub):
                    nc.tensor.matmul(
                        out=ps[:], lhsT=ones_b[:],
                        rhs=src[:, j * MM:(j + 1) * MM],
                        start=(k == 0 and j == 0),
                        stop=(k == nch - 1 and j == sub - 1),
                    )

            # ================= pass 1: sum(x), sum(x^2) =================
            for k in range(nch):
                xt = xpool.tile([P, cf], F32, tag="xt", name="xt")
                nc.sync.dma_start(out=xt[:], in_=x[:, k * cf:(k + 1) * cf])
                xb = bpool.tile([P, cf], BF16, tag="xb", name="xb")
                nc.vector.tensor_scalar(
                    out=xb[:], in0=xt[:], scalar1=1.0, scalar2=None,
                    op0=ALU.mult,
                )
                sq1 = sqpool.tile([P, cf], BF16, tag="sq", name="sq")
                nc.scalar.activation(out=sq1[:], in_=xt[:], func=ACTF.Square)
                mm_accum(ps_sx, xb, k)
                mm_accum(ps_sxx, sq1, k)

            # ---- fold partials, AllReduce #1 ---------------------------
            loc1 = smpool.tile([1, 8], F32, tag="loc1", name="loc1")
            nc.vector.memset(loc1[:], 0.0)
            nc.vector.reduce_sum(out=loc1[:, 0:1], in_=ps_sx[:],
                                 axis=mybir.AxisListType.X)
            nc.vector.reduce_sum(out=loc1[:, 1:2], in_=ps_sxx[:],
                                 axis=mybir.AxisListType.X)

            ar1_in = drpool.tile([1, 8], F32, tag="ar1_in", name="ar1_in")
            ar1_out = drpool.tile([1, 8], F32, tag="ar1_out", name="ar1_out")
            nc.sync.dma_start(out=ar1_in[:], in_=loc1[:])
            nc.gpsimd.collective_compute(
                "AllReduce", ALU.add, replica_groups=groups,
                ins=[ar1_in.opt()], outs=[ar1_out.opt()],
            )
            g1 = smpool.tile([1, 8], F32, tag="g1", name="g1")
            nc.sync.dma_start(out=g1[:], in_=ar1_out[:])
            gb1 = smpool.tile([P, 8], F32, tag="gb1", name="gb1")
            nc.gpsimd.partition_broadcast(gb1[:], g1[:])

            # ---- thresholds lo/hi (all [P,1], replicated rows) ---------
            def s_tile(tag):
                return smpool.tile([P, 1], F32, tag=tag, name=tag)

            s1g = gb1[:, 0:1]
            s2g = gb1[:, 1:2]
            mean = s_tile("mean")
            nc.vector.tensor_scalar(out=mean[:], in0=s1g, scalar1=1.0 / n_total,
                                    scalar2=None, op0=ALU.mult)
            t1 = s_tile("t1")
            nc.vector.tensor_tensor(out=t1[:], in0=s1g, in1=mean[:], op=ALU.mult)
            t2 = s_tile("t2")
            nc.vector.tensor_tensor(out=t2[:], in0=s2g, in1=t1[:], op=ALU.subtract)
            sig2 = s_tile("sig2")
            nc.vector.tensor_scalar(out=sig2[:], in0=t2[:],
                                    scalar1=1.0 / (n_total - 1.0), scalar2=EPS,
                                    op0=ALU.mult, op1=ALU.add)
            # sd = sqrt(sig2), Newton-refined: sd = 0.5*(sd0 + sig2/sd0)
            sd0 = s_tile("sd0")
            nc.scalar.sqrt(sd0[:], sig2[:])
            r0 = s_tile("r0")
            nc.vector.reciprocal(r0[:], sd0[:])
            t3 = s_tile("t3")
            nc.vector.tensor_tensor(out=t3[:], in0=sig2[:], in1=r0[:], op=ALU.mult)
            t4 = s_tile("t4")
            nc.vector.tensor_tensor(out=t4[:], in0=sd0[:], in1=t3[:], op=ALU.add)
            s4 = s_tile("s4")
            nc.vector.tensor_scalar(out=s4[:], in0=t4[:], scalar1=0.5 * THRES,
                                    scalar2=None, op0=ALU.mult)
            lo = s_tile("lo")
            nc.vector.tensor_tensor(out=lo[:], in0=mean[:], in1=s4[:],
                                    op=ALU.subtract)
            hi = s_tile("hi")
            nc.vector.tensor_tensor(out=hi[:], in0=mean[:], in1=s4[:], op=ALU.add)

            # ===== pass 2: sum(c), sum(c^2), n_lo, n_hi =================
            for k in range(nch):
                xt = xpool.tile([P, cf], F32, tag="xt", name="xt")
                nc.sync.dma_start(out=xt[:], in_=x[:, k * cf:(k + 1) * cf])
                ct = bpool.tile([P, cf], BF16, tag="ct", name="ct")
                nc.vector.tensor_scalar(
                    out=ct[:], in0=xt[:], scalar1=lo[:, 0:1], scalar2=hi[:, 0:1],
                    op0=ALU.max, op1=ALU.min,
                )
                ilo = bpool.tile([P, cf], BF16, tag="ilo", name="ilo")
                nc.vector.tensor_scalar(
                    out=ilo[:], in0=xt[:], scalar1=lo[:, 0:1], scalar2=None,
                    op0=ALU.is_le,
                )
                ihi = bpool.tile([P, cf], BF16, tag="ihi", name="ihi")
                nc.vector.tensor_scalar(
                    out=ihi[:], in0=xt[:], scalar1=hi[:, 0:1], scalar2=None,
                    op0=ALU.is_ge,
                )
                sq2 = sqpool.tile([P, cf], BF16, tag="sq", name="sq")
                nc.scalar.activation(out=sq2[:], in_=ct[:], func=ACTF.Square)
                mm_accum(ps_sc, ct, k)
                mm_accum(ps_scc, sq2, k)
                mm_accum(ps_nlo, ilo, k)
                mm_accum(ps_nhi, ihi, k)

            # ---- fold partials, AllReduce #2 ---------------------------
            loc2 = smpool.tile([1, 8], F32, tag="loc2", name="loc2")
            nc.vector.memset(loc2[:], 0.0)
            nc.vector.reduce_sum(out=loc2[:, 0:1], in_=ps_sc[:],
                                 axis=mybir.AxisListType.X)
            nc.vector.reduce_sum(out=loc2[:, 1:2], in_=ps_scc[:],
                                 axis=mybir.AxisListType.X)
            nc.vector.reduce_sum(out=loc2[:, 2:3], in_=ps_nlo[:],
                                 axis=mybir.AxisListType.X)
            nc.vector.reduce_sum(out=loc2[:, 3:4], in_=ps_nhi[:],
                                 axis=mybir.AxisListType.X)

            ar2_in = drpool.tile([1, 8], F32, tag="ar2_in", name="ar2_in")
            ar2_out = drpool.tile([1, 8], F32, tag="ar2_out", name="ar2_out")
            nc.sync.dma_start(out=ar2_in[:], in_=loc2[:])
            nc.gpsimd.collective_compute(
                "AllReduce", ALU.add, replica_groups=groups,
                ins=[ar2_in.opt()], outs=[ar2_out.opt()],
            )
            g2 = smpool.tile([1, 8], F32, tag="g2", name="g2")
            nc.sync.dma_start(out=g2[:], in_=ar2_out[:])
            gb2 = smpool.tile([P, 8], F32, tag="gb2", name="gb2")
            nc.gpsimd.partition_broadcast(gb2[:], g2[:])

            # ---- masked moments -> EMA -> affine coefficients ----------
            sc_g = gb2[:, 0:1]
            scc_g = gb2[:, 1:2]
            nlo_g = gb2[:, 2:3]
            nhi_g = gb2[:, 3:4]

            u = s_tile("u")
            nc.vector.tensor_tensor(out=u[:], in0=nlo_g, in1=nhi_g, op=ALU.add)
            cnt = s_tile("cnt")
            nc.vector.tensor_scalar(out=cnt[:], in0=u[:], scalar1=n_total,
                                    scalar2=-1.0, op0=ALU.subtract, op1=ALU.mult)
            w1 = s_tile("w1")
            nc.vector.tensor_tensor(out=w1[:], in0=lo[:], in1=nlo_g, op=ALU.mult)
            w2 = s_tile("w2")
            nc.vector.tensor_tensor(out=w2[:], in0=hi[:], in1=nhi_g, op=ALU.mult)
            w3 = s_tile("w3")
            nc.vector.tensor_tensor(out=w3[:], in0=w1[:], in1=w2[:], op=ALU.add)
            s1m = s_tile("s1m")
            nc.vector.tensor_tensor(out=s1m[:], in0=sc_g, in1=w3[:],
                                    op=ALU.subtract)
            lo2 = s_tile("lo2")
            nc.vector.tensor_tensor(out=lo2[:], in0=lo[:], in1=lo[:], op=ALU.mult)
            hi2 = s_tile("hi2")
            nc.vector.tensor_tensor(out=hi2[:], in0=hi[:], in1=hi[:], op=ALU.mult)
            v1 = s_tile("v1")
            nc.vector.tensor_tensor(out=v1[:], in0=lo2[:], in1=nlo_g, op=ALU.mult)
            v2 = s_tile("v2")
            nc.vector.tensor_tensor(out=v2[:], in0=hi2[:], in1=nhi_g, op=ALU.mult)
            v3 = s_tile("v3")
            nc.vector.tensor_tensor(out=v3[:], in0=v1[:], in1=v2[:], op=ALU.add)
            s2m = s_tile("s2m")
            nc.vector.tensor_tensor(out=s2m[:], in0=scc_g, in1=v3[:],
                                    op=ALU.subtract)

            rc = s_tile("rc")
            nc.vector.reciprocal(rc[:], cnt[:])
            pmean = s_tile("pmean")
            nc.vector.tensor_tensor(out=pmean[:], in0=s1m[:], in1=rc[:],
                                    op=ALU.mult)
            pt = s_tile("pt")
            nc.vector.tensor_tensor(out=pt[:], in0=pmean[:], in1=s1m[:],
                                    op=ALU.mult)
            pt2 = s_tile("pt2")
            nc.vector.tensor_tensor(out=pt2[:], in0=s2m[:], in1=pt[:],
                                    op=ALU.subtract)
            cm1 = s_tile("cm1")
            nc.vector.tensor_scalar(out=cm1[:], in0=cnt[:], scalar1=-1.0,
                                    scalar2=None, op0=ALU.add)
            rc1 = s_tile("rc1")
            nc.vector.reciprocal(rc1[:], cm1[:])
            pvar = s_tile("pvar")
            nc.vector.tensor_tensor(out=pvar[:], in0=pt2[:], in1=rc1[:],
                                    op=ALU.mult)

            runm = s_tile("runm")
            nc.vector.tensor_scalar(out=runm[:], in0=pmean[:],
                                    scalar1=1.0 - ALPHA, scalar2=None,
                                    op0=ALU.mult)
            runv = s_tile("runv")
            nc.vector.tensor_scalar(out=runv[:], in0=pvar[:],
                                    scalar1=1.0 - ALPHA, scalar2=ALPHA,
                                    op0=ALU.mult, op1=ALU.add)
            q = s_tile("q")
            nc.vector.tensor_scalar(out=q[:], in0=runv[:], scalar1=EPS,
                                    scalar2=None, op0=ALU.add)
            # rstd = 1/sqrt(q) = refined_sqrt(q) * (1/q)
            qs0 = s_tile("qs0")
            nc.scalar.sqrt(qs0[:], q[:])
            qr0 = s_tile("qr0")
            nc.vector.reciprocal(qr0[:], qs0[:])
            qt = s_tile("qt")
            nc.vector.tensor_tensor(out=qt[:], in0=q[:], in1=qr0[:], op=ALU.mult)
            qt2 = s_tile("qt2")
            nc.vector.tensor_tensor(out=qt2[:], in0=qs0[:], in1=qt[:], op=ALU.add)
            sdr = s_tile("sdr")
            nc.vector.tensor_scalar(out=sdr[:], in0=qt2[:], scalar1=0.5,
                                    scalar2=None, op0=ALU.mult)
            rq = s_tile("rq")
            nc.vector.reciprocal(rq[:], q[:])
            rstd = s_tile("rstd")
            nc.vector.tensor_tensor(out=rstd[:], in0=sdr[:], in1=rq[:],
                                    op=ALU.mult)
            a_co = s_tile("a_co")
            nc.vector.tensor_tensor(out=a_co[:], in0=rstd[:], in1=gamma_b[:],
                                    op=ALU.mult)
            rma = s_tile("rma")
            nc.vector.tensor_tensor(out=rma[:], in0=runm[:], in1=a_co[:],
                                    op=ALU.mult)
            b_co = s_tile("b_co")
            nc.vector.tensor_tensor(out=b_co[:], in0=beta_b[:], in1=rma[:],
                                    op=ALU.subtract)

            # ================= pass 3: out = a*x + b ====================
            for k in range(nch):
                xt = xpool.tile([P, cf], F32, tag="xt", name="xt")
                nc.sync.dma_start(out=xt[:], in_=x[:, k * cf:(k + 1) * cf])
                ot = opool.tile([P, cf], F32, tag="ot", name="ot")
                nc.scalar.activation(
                    out=ot[:], in_=xt[:], func=ACTF.Identity,
                    bias=b_co[:, 0:1], scale=a_co[:, 0:1],
                )
                nc.sync.dma_start(out=out[:, k * cf:(k + 1) * cf], in_=ot[:])

    nc.compile()
    return nc


_BUILT = {}


def _get_built(f_per_part, cf, n_cores=N_CORES):
    key = (f_per_part, cf, n_cores)
    if key not in _BUILT:
        _BUILT[key] = build_bass(f_per_part, cf, n_cores)
    return _BUILT[key]


def run(xorig: np.ndarray, gamma: np.ndarray, beta: np.ndarray,
        f_per_part: int = F_FULL, cf: int = CF_FULL, **spmd_kwargs):
    """Shard, run on 8 cores, gather. Returns (output, BassKernelResults)."""
    xorig = np.ascontiguousarray(np.asarray(xorig, dtype=np.float32))
    rows, cols = xorig.shape
    assert rows % N_CORES == 0
    g = np.asarray(gamma, dtype=np.float32).reshape(1, 1)
    b = np.asarray(beta, dtype=np.float32).reshape(1, 1)

    nc = _get_built(f_per_part, cf)

    shard_rows = rows // N_CORES
    in_maps = []
    for i in range(N_CORES):
        shard = xorig[i * shard_rows:(i + 1) * shard_rows].reshape(P, f_per_part)
        in_maps.append({"x": shard, "gamma": g, "beta": b})

    res = run_bass_kernel_spmd(nc, in_maps, core_ids=list(range(N_CORES)),
                               **spmd_kwargs)
    outs = [res.results[i]["out"].reshape(shard_rows, cols)
            for i in range(N_CORES)]
    return np.concatenate(outs, axis=0), res


def kernel(xorig, gamma, beta):
    out, _ = run(np.asarray(xorig), np.asarray(gamma), np.asarray(beta))
    return out
